# revision 38
# baseline (speedup 1.0000x reference)
"""Masked multi-head attention (B=4, S=2048, H=16, d_k=64) on 8 TRN2 NeuronCores.

Sharding: core c handles batch b = c//2 and head-group hg = c%2 (8 heads each).
Per core (layouts avoid all on-chip transposes):
  scoresT[k,q] = K @ Q^T        bf16, two heads row-packed per 128-partition pair
  E = exp(scoresT/8)*maskT      ACT exp PSUM->bf16 (fast 2-elem/cycle path);
                                mask-mult on DVE (4/5) + GPSIMD (1/5)
  outT,Z = [V|ones] @ E         bf16 matmuls accumulated over 16 k-tiles; the
                                ones columns emit Z on PSUM rows 64-127
  out = outT * exp(-ln Z)       1/Z on ACT (Ln+Exp; ACT has slack), with the
                                whole finalize deferred into the next pair's
                                k-loop so it never head-of-line blocks the
                                engines; output stored bf16 (halves the
                                output DMA), upcast to fp32 on host.
"""
import sys
sys.path.insert(0, "/opt/trn_rl_repo")
import numpy as np
import ml_dtypes
import concourse.bass as bass
import concourse.tile as tile
import concourse.mybir as mybir
from concourse import bacc
from concourse import bass_utils

BF16 = mybir.dt.bfloat16
F32 = mybir.dt.float32
S = 2048; DK = 64; HPC = 8; N_CORES = 8; QW = 512; P = 128
GPSIMD_EVERY = 5
FINALIZE = "lnexp"  # recip | divide | lnexp

def build_program(s=S, hpc=HPC, reps=1):
    kt_n = s // P
    qt_n = s // QW
    pairs = hpc // 2
    hd = hpc * DK
    nc = bacc.Bacc("TRN2", debug=False)
    qT = nc.dram_tensor("qT", [hd, s], BF16, kind="ExternalInput").ap()
    kT = nc.dram_tensor("kT", [hd, s], BF16, kind="ExternalInput").ap()
    v = nc.dram_tensor("v", [s, hd], BF16, kind="ExternalInput").ap()
    mT = nc.dram_tensor("mT", [s, s], BF16, kind="ExternalInput").ap()
    outT = nc.dram_tensor("outT", [hd, s], BF16, kind="ExternalOutput").ap()
    Exp = mybir.ActivationFunctionType.Exp
    Log = mybir.ActivationFunctionType.Ln
    with tile.TileContext(nc) as tc:
        with (
            tc.tile_pool(name="resident", bufs=1) as resident,
            tc.tile_pool(name="maskp", bufs=2) as maskp,
            tc.tile_pool(name="erawp", bufs=3) as erawp,
            tc.tile_pool(name="ep", bufs=3) as ep,
            tc.tile_pool(name="rcpp", bufs=2) as rcpp,
            tc.tile_pool(name="osbp", bufs=2) as osbp,
            tc.tile_pool(name="psum_s", bufs=2, space="PSUM") as psum_s,
            tc.tile_pool(name="psum_o", bufs=2, space="PSUM") as psum_o,
        ):
            # first mask window first so qt=0 mask-mults don't starve
            m_sb0 = maskp.tile([P, kt_n * QW], BF16, tag="m")
            for kt in range(kt_n):
                nc.sync.dma_start(m_sb0[:, kt * QW:(kt + 1) * QW],
                                  mT[kt * P:(kt + 1) * P, 0:QW])

            qT_sb = resident.tile([P, pairs * s], BF16)
            kT_sb = resident.tile([P, pairs * s], BF16)
            for p in range(pairs):
                nc.sync.dma_start(qT_sb[:, p * s:(p + 1) * s], qT[p * P:(p + 1) * P, :])
                nc.sync.dma_start(kT_sb[:, p * s:(p + 1) * s], kT[p * P:(p + 1) * P, :])
            v_sb = resident.tile([P, hpc * kt_n * P], BF16)
            v_sb3 = v_sb.rearrange("p (t e) -> p t e", e=P)
            nc.gpsimd.memset(v_sb3[:, :, 64:128], 1.0)
            v_src = v.rearrange("(kt p) c -> p kt c", p=P)
            for h in range(hpc):
                dst = v_sb[:, h * kt_n * P:(h + 1) * kt_n * P]
                dst3 = dst.rearrange("p (kt e) -> p kt e", e=P)
                nc.sync.dma_start(dst3[:, :, 0:64], v_src[:, :, h * DK:(h + 1) * DK])
            pending = None

            def _finalize(o_ps, hA, hB, qt):
                o_sb = osbp.tile([64, 2 * QW], BF16)
                if FINALIZE == "recip":
                    # rcp = 1/Z (Z replicated on psum rows 64-127)
                    rcp = rcpp.tile([64, 2 * QW], F32, tag="rcp")
                    nc.vector.reciprocal(rcp[:], o_ps[64:128, :])
                    nc.vector.tensor_mul(o_sb[:], o_ps[0:64, :], rcp[:])
                elif FINALIZE == "divide":
                    nc.vector.tensor_tensor(o_sb[:], o_ps[0:64, :],
                                            o_ps[64:128, :],
                                            mybir.AluOpType.divide)
                else:  # lnexp on ACT (tables swap, but ACT has slack)
                    lnz = rcpp.tile([64, 2 * QW], F32, tag="lnz")
                    nc.scalar.activation(lnz[:], o_ps[64:128, :], Log)
                    rcp = rcpp.tile([64, 2 * QW], F32, tag="rcp")
                    nc.scalar.activation(rcp[:], lnz[:], Exp, scale=-1.0)
                    nc.vector.tensor_mul(o_sb[:], o_ps[0:64, :], rcp[:])
                for h, half in ((hA, slice(0, QW)), (hB, slice(QW, 2 * QW))):
                    nc.sync.dma_start(
                        outT[h * DK:(h + 1) * DK, qt * QW:(qt + 1) * QW],
                        o_sb[:, half])

            for rep in range(reps):
              for qt in range(qt_n):
                  if rep == 0 and qt == 0:
                      m_sb = m_sb0
                  else:
                      m_sb = maskp.tile([P, kt_n * QW], BF16, tag="m")
                      for kt in range(kt_n):
                          nc.sync.dma_start(
                              m_sb[:, kt * QW:(kt + 1) * QW],
                              mT[kt * P:(kt + 1) * P, qt * QW:(qt + 1) * QW])
                  for p in range(pairs):
                      hA, hB = 2 * p, 2 * p + 1
                      o_ps = psum_o.tile([P, 2 * QW], F32, tag="ops")
                      for kt in range(kt_n):
                          if kt == 5 and pending is not None:
                              _finalize(*pending)
                              pending = None
                          s_ps = psum_s.tile([P, 2 * QW], F32)
                          nc.tensor.matmul(
                              s_ps[:, 0:QW],
                              lhsT=kT_sb[0:64, p * s + kt * P: p * s + (kt + 1) * P],
                              rhs=qT_sb[0:64, p * s + qt * QW: p * s + (qt + 1) * QW],
                              start=True, stop=True)
                          nc.tensor.matmul(
                              s_ps[:, QW:2 * QW],
                              lhsT=kT_sb[64:128, p * s + kt * P: p * s + (kt + 1) * P],
                              rhs=qT_sb[64:128, p * s + qt * QW: p * s + (qt + 1) * QW],
                              start=True, stop=True)
                          e_raw = erawp.tile([P, 2 * QW], BF16)
                          nc.scalar.activation(e_raw[:], s_ps[:], Exp, scale=0.125)
                          e = ep.tile([P, 2 * QW], BF16)
                          msl = m_sb[:, kt * QW:(kt + 1) * QW]
                          eng = nc.gpsimd if kt % GPSIMD_EVERY == GPSIMD_EVERY - 1 else nc.vector
                          eng.tensor_mul(e[:, 0:QW], e_raw[:, 0:QW], msl)
                          eng.tensor_mul(e[:, QW:2 * QW], e_raw[:, QW:2 * QW], msl)
                          vofsA = (hA * kt_n + kt) * P
                          vofsB = (hB * kt_n + kt) * P
                          nc.tensor.matmul(
                              o_ps[:, 0:QW], lhsT=v_sb[:, vofsA:vofsA + P],
                              rhs=e[:, 0:QW],
                              start=(kt == 0), stop=(kt == kt_n - 1))
                          nc.tensor.matmul(
                              o_ps[:, QW:2 * QW], lhsT=v_sb[:, vofsB:vofsB + P],
                              rhs=e[:, QW:2 * QW],
                              start=(kt == 0), stop=(kt == kt_n - 1))
                      pending = (o_ps, hA, hB, qt)
              if pending is not None:
                  _finalize(*pending)
                  pending = None
    nc.compile()
    return nc

def _prep_in_maps(query, key, value, mask):
    query = np.asarray(query, dtype=np.float32)
    key = np.asarray(key, dtype=np.float32)
    value = np.asarray(value, dtype=np.float32)
    mask = np.asarray(mask)
    B = query.shape[0]
    bf16 = ml_dtypes.bfloat16
    hd = HPC * DK
    mTs = [np.ascontiguousarray(mask[b, 0].T).astype(bf16) for b in range(B)]
    in_maps = []
    for c in range(N_CORES):
        b, hg = divmod(c, 2)
        cols = slice(hg * hd, (hg + 1) * hd)
        in_maps.append({
            "qT": np.ascontiguousarray(query[b][:, cols].T).astype(bf16),
            "kT": np.ascontiguousarray(key[b][:, cols].T).astype(bf16),
            "v": value[b][:, cols].astype(bf16),
            "mT": mTs[b],
        })
    return in_maps


_PROG = None


def _get_prog():
    global _PROG
    if _PROG is None:
        _PROG = build_program()
    return _PROG


def _unshard(results, B, s, D):
    hd = HPC * DK
    out = np.empty((B, s, D), np.float32)
    for c in range(N_CORES):
        b, hg = divmod(c, 2)
        out[b][:, hg * hd:(hg + 1) * hd] = results[c]["outT"].T.astype(np.float32)
    return out


def kernel(query, key, value, mask):
    global LAST_RESULTS
    B, s, D = np.asarray(query).shape
    in_maps = _prep_in_maps(query, key, value, mask)
    nc = _get_prog()
    res = bass_utils.run_bass_kernel_spmd(
        nc, in_maps, core_ids=list(range(N_CORES)), trace=False)
    LAST_RESULTS = res
    return _unshard(res.results, B, s, D)


LAST_RESULTS = None


# revision 41
# speedup vs baseline: 1.4715x; 1.4715x over previous
"""Masked multi-head attention (B=4, S=2048, H=16, d_k=64) on 8 TRN2 NeuronCores.

Sharding: core c handles batch b = c//2 and head-group hg = c%2 (8 heads each).
Per core (layouts avoid all on-chip transposes):
  scoresT[k,q] = K @ Q^T        bf16, two heads row-packed per 128-partition pair
  E = exp(scoresT/8)*maskT      ACT exp PSUM->bf16 (fast 2-elem/cycle path);
                                mask-mult on DVE (4/5) + GPSIMD (1/5)
  outT,Z = [V|ones] @ E         bf16 matmuls accumulated over 16 k-tiles; the
                                ones columns emit Z on PSUM rows 64-127
  out = outT * exp(-ln Z)       1/Z on ACT (Ln+Exp; ACT has slack), with the
                                whole finalize deferred into the next pair's
                                k-loop so it never head-of-line blocks the
                                engines; output stored bf16 (halves the
                                output DMA), upcast to fp32 on host.
"""
import sys
sys.path.insert(0, "/opt/trn_rl_repo")
import numpy as np
import ml_dtypes
import concourse.bass as bass
import concourse.tile as tile
import concourse.mybir as mybir
from concourse import bacc
from concourse import bass_utils

BF16 = mybir.dt.bfloat16
F32 = mybir.dt.float32
S = 2048; DK = 64; HPC = 8; N_CORES = 8; QW = 512; P = 128
GPSIMD_EVERY = 5
FINALIZE = "lnexp"  # recip | divide | lnexp

def build_program(s=S, hpc=HPC, reps=1):
    kt_n = s // P
    qt_n = s // QW
    pairs = hpc // 2
    hd = hpc * DK
    nc = bacc.Bacc("TRN2", debug=False)
    qT = nc.dram_tensor("qT", [hd, s], BF16, kind="ExternalInput").ap()
    kT = nc.dram_tensor("kT", [hd, s], BF16, kind="ExternalInput").ap()
    v = nc.dram_tensor("v", [s, hd], BF16, kind="ExternalInput").ap()
    mT = nc.dram_tensor("mT", [s, s], BF16, kind="ExternalInput").ap()
    outT = nc.dram_tensor("outT", [hd, s], BF16, kind="ExternalOutput").ap()
    Exp = mybir.ActivationFunctionType.Exp
    Log = mybir.ActivationFunctionType.Ln
    with tile.TileContext(nc) as tc:
        with (
            tc.tile_pool(name="resident", bufs=1) as resident,
            tc.tile_pool(name="maskp", bufs=2) as maskp,
            tc.tile_pool(name="erawp", bufs=4) as erawp,
            tc.tile_pool(name="ep", bufs=4) as ep,
            tc.tile_pool(name="rcpp", bufs=2) as rcpp,
            tc.tile_pool(name="osbp", bufs=2) as osbp,
            tc.tile_pool(name="psum_s", bufs=2, space="PSUM") as psum_s,
            tc.tile_pool(name="psum_o", bufs=2, space="PSUM") as psum_o,
        ):
            # first mask window first so qt=0 mask-mults don't starve
            mTr = mT.rearrange("(t p) q -> p t q", p=P)
            m_sb0 = maskp.tile([P, kt_n * QW], BF16, tag="m")
            nc.sync.dma_start(m_sb0.rearrange("p (t w) -> p t w", w=QW),
                              mTr[:, :, 0:QW])

            qT_sb = resident.tile([P, pairs * s], BF16)
            kT_sb = resident.tile([P, pairs * s], BF16)
            for p in range(pairs):
                nc.sync.dma_start(qT_sb[:, p * s:(p + 1) * s], qT[p * P:(p + 1) * P, :])
                nc.sync.dma_start(kT_sb[:, p * s:(p + 1) * s], kT[p * P:(p + 1) * P, :])
            v_sb = resident.tile([P, hpc * kt_n * P], BF16)
            v_sb3 = v_sb.rearrange("p (t e) -> p t e", e=P)
            nc.gpsimd.memset(v_sb3[:, :, 64:128], 1.0)
            v_src = v.rearrange("(kt p) c -> p kt c", p=P)
            for h in range(hpc):
                dst = v_sb[:, h * kt_n * P:(h + 1) * kt_n * P]
                dst3 = dst.rearrange("p (kt e) -> p kt e", e=P)
                nc.sync.dma_start(dst3[:, :, 0:64], v_src[:, :, h * DK:(h + 1) * DK])
            pending = None

            def _finalize(o_ps, hA, hB, qt):
                o_sb = osbp.tile([64, 2 * QW], BF16)
                if FINALIZE == "recip":
                    # rcp = 1/Z (Z replicated on psum rows 64-127)
                    rcp = rcpp.tile([64, 2 * QW], F32, tag="rcp")
                    nc.vector.reciprocal(rcp[:], o_ps[64:128, :])
                    nc.vector.tensor_mul(o_sb[:], o_ps[0:64, :], rcp[:])
                elif FINALIZE == "divide":
                    nc.vector.tensor_tensor(o_sb[:], o_ps[0:64, :],
                                            o_ps[64:128, :],
                                            mybir.AluOpType.divide)
                else:  # lnexp on ACT (tables swap, but ACT has slack)
                    lnz = rcpp.tile([64, 2 * QW], F32, tag="lnz")
                    nc.scalar.activation(lnz[:], o_ps[64:128, :], Log)
                    rcp = rcpp.tile([64, 2 * QW], F32, tag="rcp")
                    nc.scalar.activation(rcp[:], lnz[:], Exp, scale=-1.0)
                    nc.vector.tensor_mul(o_sb[:], o_ps[0:64, :], rcp[:])
                for h, half in ((hA, slice(0, QW)), (hB, slice(QW, 2 * QW))):
                    nc.sync.dma_start(
                        outT[h * DK:(h + 1) * DK, qt * QW:(qt + 1) * QW],
                        o_sb[:, half])

            for rep in range(reps):
              for qt in range(qt_n):
                  if rep == 0 and qt == 0:
                      m_sb = m_sb0
                  else:
                      m_sb = maskp.tile([P, kt_n * QW], BF16, tag="m")
                      nc.sync.dma_start(
                          m_sb.rearrange("p (t w) -> p t w", w=QW),
                          mTr[:, :, qt * QW:(qt + 1) * QW])
                  for p in range(pairs):
                      hA, hB = 2 * p, 2 * p + 1
                      o_ps = psum_o.tile([P, 2 * QW], F32, tag="ops")
                      for kt in range(kt_n):
                          if kt == 5 and pending is not None:
                              _finalize(*pending)
                              pending = None
                          s_ps = psum_s.tile([P, 2 * QW], F32)
                          nc.tensor.matmul(
                              s_ps[:, 0:QW],
                              lhsT=kT_sb[0:64, p * s + kt * P: p * s + (kt + 1) * P],
                              rhs=qT_sb[0:64, p * s + qt * QW: p * s + (qt + 1) * QW],
                              start=True, stop=True)
                          nc.tensor.matmul(
                              s_ps[:, QW:2 * QW],
                              lhsT=kT_sb[64:128, p * s + kt * P: p * s + (kt + 1) * P],
                              rhs=qT_sb[64:128, p * s + qt * QW: p * s + (qt + 1) * QW],
                              start=True, stop=True)
                          e_raw = erawp.tile([P, 2 * QW], BF16)
                          nc.scalar.activation(e_raw[:], s_ps[:], Exp, scale=0.125)
                          e = ep.tile([P, 2 * QW], BF16)
                          msl = m_sb[:, kt * QW:(kt + 1) * QW]
                          eng = nc.gpsimd if kt % GPSIMD_EVERY == GPSIMD_EVERY - 1 else nc.vector
                          eng.tensor_mul(e[:, 0:QW], e_raw[:, 0:QW], msl)
                          eng.tensor_mul(e[:, QW:2 * QW], e_raw[:, QW:2 * QW], msl)
                          vofsA = (hA * kt_n + kt) * P
                          vofsB = (hB * kt_n + kt) * P
                          nc.tensor.matmul(
                              o_ps[:, 0:QW], lhsT=v_sb[:, vofsA:vofsA + P],
                              rhs=e[:, 0:QW],
                              start=(kt == 0), stop=(kt == kt_n - 1))
                          nc.tensor.matmul(
                              o_ps[:, QW:2 * QW], lhsT=v_sb[:, vofsB:vofsB + P],
                              rhs=e[:, QW:2 * QW],
                              start=(kt == 0), stop=(kt == kt_n - 1))
                      pending = (o_ps, hA, hB, qt)
              if pending is not None:
                  _finalize(*pending)
                  pending = None
    nc.compile()
    return nc

def _prep_in_maps(query, key, value, mask):
    query = np.asarray(query, dtype=np.float32)
    key = np.asarray(key, dtype=np.float32)
    value = np.asarray(value, dtype=np.float32)
    mask = np.asarray(mask)
    B = query.shape[0]
    bf16 = ml_dtypes.bfloat16
    hd = HPC * DK
    mTs = [np.ascontiguousarray(mask[b, 0].T).astype(bf16) for b in range(B)]
    in_maps = []
    for c in range(N_CORES):
        b, hg = divmod(c, 2)
        cols = slice(hg * hd, (hg + 1) * hd)
        in_maps.append({
            "qT": np.ascontiguousarray(query[b][:, cols].T).astype(bf16),
            "kT": np.ascontiguousarray(key[b][:, cols].T).astype(bf16),
            "v": value[b][:, cols].astype(bf16),
            "mT": mTs[b],
        })
    return in_maps


_PROG = None


def _get_prog():
    global _PROG
    if _PROG is None:
        _PROG = build_program()
    return _PROG


def _unshard(results, B, s, D):
    hd = HPC * DK
    out = np.empty((B, s, D), np.float32)
    for c in range(N_CORES):
        b, hg = divmod(c, 2)
        out[b][:, hg * hd:(hg + 1) * hd] = results[c]["outT"].T.astype(np.float32)
    return out


def kernel(query, key, value, mask):
    global LAST_RESULTS
    B, s, D = np.asarray(query).shape
    in_maps = _prep_in_maps(query, key, value, mask)
    nc = _get_prog()
    res = bass_utils.run_bass_kernel_spmd(
        nc, in_maps, core_ids=list(range(N_CORES)), trace=False)
    LAST_RESULTS = res
    return _unshard(res.results, B, s, D)


LAST_RESULTS = None
